# revision 2
# baseline (speedup 1.0000x reference)
"""Trainium2 Bass kernel: sigmoid(rowdot(tanh(x1@W.T+b), tanh(x2@W.T+b))).

Sharding: pure data-parallel over batch across 8 NeuronCores. Per-core
shapes hardcoded (B=65536 total -> 8192 rows/core, D_IN=1024, D_PROJ=128).
x1/x2 shards are fused into one device tensor "xc" [2*8192, 1024] cast to
fp16 on the host; W.T (fp16), bias (fp32), identity and all-ones (fp16)
are tiny host-precomputed inputs.

fp16 halves HBM traffic vs fp32 (32 MiB/core, ~93 us at the ~358 GB/s
per-core limit) while keeping enough mantissa (11 bits) that end-to-end
max rel err stays ~1e-2 under the 2e-2 gate. With fp32 the kernel was
DMA-bound at ~187 us; at fp16 the bottleneck moves to the PE if it also
does all transposes, so transposes are split:

  - chunks 0..K_NAT-1 of each row-tile load naturally ([128p, g, 768d])
    and are transposed on the PE (fp16 transpose = 1 cyc/row), copied
    PSUM->SBUF by DVE/ACT alternately;
  - chunks K_NAT..7 (columns 768..1024) load via the DMA XBAR transpose
    (dma_start_transpose, 16x128 source tiles, 2-byte dtype) directly
    into the transposed SBUF layout, costing DMA ~14ns/4KiB-tile but
    zero PE/DVE/ACT work.

With K_DMAT=2 both DMA (~99 us) and PE (matmul 55 + transpose 41 +
reduce 3 us) are balanced near their rooflines.

Per-core dataflow per 512-row batch tile (256-row tiles at both ends to
shorten pipeline ramp-in and drain), all-fp16 compute path:
  1. natural x loads + XBAR-transposed tail-chunk loads (SP queue).
  2. PE transpose fp16 -> PSUM for natural chunks; DVE/ACT copy to SBUF.
  3. PE matmul fp16 (1 cyc/row): oT[j,b] += Wt_k.T @ xT_k, fp32 PSUM.
  4. ACT: t = tanh(oT + bias) -> fp16 SBUF.
  5. DVE: prod = t1 * t2 (fp16).
  6. PE: sim = ones.T @ prod -> fp32 PSUM (partition reduction).
  7. ACT sigmoid -> fp32; 2 KiB output DMA on a rotating partition.

Software pipelining (as in the fp32 version): tile i's matmuls are
emitted interleaved into tile i+1's transpose stream (keeps the PE HAM
clock-gate warm, no phase barriers), and tile i's reduce rides inside
tile i+2's transpose phase. PSUM: 5 transpose tiles + 3 matmul banks.
"""

import numpy as np

import concourse.bacc as bacc
import concourse.mybir as mybir
import concourse.tile as tile
from concourse.bass_utils import run_bass_kernel_spmd

N_CORES = 8
B_TOTAL = 65536
BSH = B_TOTAL // N_CORES  # 8192 rows per core
D_IN = 1024
D_PROJ = 128
P = 128
BT = 512                 # batch tile (matmul moving dim)
NBT = BSH // BT          # 16 batch tiles per core
KC = D_IN // P           # 8 contraction chunks
K_DMAT = 2               # chunks per branch loaded via DMA XBAR transpose
K_NAT = KC - K_DMAT      # chunks via natural load + PE transpose
DN = K_NAT * P           # natural-load columns

F32 = mybir.dt.float32
F16 = mybir.dt.float16


def _build_module():
    nc = bacc.Bacc("TRN2", target_bir_lowering=False, debug=False)

    xc = nc.dram_tensor("xc", [2 * BSH, D_IN], F16, kind="ExternalInput").ap()
    x1 = xc[:BSH]
    x2 = xc[BSH:]
    wt = nc.dram_tensor("wt", [D_IN, D_PROJ], F16, kind="ExternalInput").ap()
    bias = nc.dram_tensor("bias", [P, 1], F32, kind="ExternalInput").ap()
    ident = nc.dram_tensor("ident", [P, P], F16, kind="ExternalInput").ap()
    ones = nc.dram_tensor("ones", [P, P], F16, kind="ExternalInput").ap()
    out = nc.dram_tensor("out", [BSH], F32, kind="ExternalOutput").ap()

    outf = out  # [BSH]
    x1n = x1.rearrange("(g p) d -> p g d", p=P)  # [128, BSH//128, D_IN]
    x2n = x2.rearrange("(g p) d -> p g d", p=P)

    with tile.TileContext(nc) as tc:
        with (
            tc.tile_pool(name="consts", bufs=1) as cpool,
            tc.tile_pool(name="xnat", bufs=3) as natpool,
            tc.tile_pool(name="xt", bufs=2) as xtpool,
            tc.tile_pool(name="acts", bufs=2) as apool,
            tc.tile_pool(name="ptr", bufs=5, space="PSUM") as trpool,
            tc.tile_pool(name="po", bufs=3, space="PSUM") as opool,
        ):
            # identity first (32 KiB) -- it gates the first transposes; the
            # 256 KiB W.T load is emitted after the first x-tile loads.
            ident_sb = cpool.tile([P, P], F16, tag="ident")
            nc.sync.dma_start(out=ident_sb, in_=ident)
            wt_sb = cpool.tile([P, KC, D_PROJ], F16, tag="wt")
            bias_sb = cpool.tile([P, 1], F32, tag="bias")
            ones_sb = cpool.tile([P, P], F16, tag="ones")

            # Work list: (row0, nrows). First and last 512-row blocks are
            # split into 256-row subtiles: small first tiles shorten the
            # pipeline ramp-in, small last tiles shorten the drain.
            h = BT // 2
            tiles = [(0, h), (h, h)]
            tiles += [(t * BT, BT) for t in range(1, NBT - 1)]
            last = (NBT - 1) * BT
            tiles += [(last, h), (last + h, h)]

            # Tail of tile i (rowdot reduce + sigmoid + store) is emitted
            # in the middle of tile i+1's transpose phase so PE never
            # waits on the tanh->mul chain.
            pending = []

            def flush_pending():
                while pending:
                    prod_p, row0_p, nr_p, idx_p = pending.pop(0)
                    psim = opool.tile([P, nr_p], F32, name="psim", tag="po")
                    nc.tensor.matmul(
                        psim,
                        ones_sb,
                        prod_p,
                        start=True,
                        stop=True,
                        skip_group_check=True,
                    )
                    sig = apool.tile([P, nr_p], F32, tag="sig")
                    nc.scalar.activation(
                        sig, psim, mybir.ActivationFunctionType.Sigmoid
                    )
                    row = (idx_p * 4) % P  # rotate partition -> spread DMA engines
                    nc.scalar.dma_start(
                        out=outf[row0_p:row0_p + nr_p].rearrange(
                            "(a n) -> a n", a=1
                        ),
                        in_=sig[row:row + 1, :],
                    )

            def tr_chunk(xn, xt_sb, g_cnt, nrows, k, eng):
                ps = trpool.tile([P, nrows], F16, tag="tr")
                for g in range(g_cnt):
                    nc.tensor.transpose(
                        ps[:, g * P:(g + 1) * P],
                        xn[:, g, k * P:(k + 1) * P],
                        ident_sb,
                    )
                if eng == 0:
                    nc.vector.tensor_copy(xt_sb[:, k, :], ps)
                else:
                    nc.scalar.copy(xt_sb[:, k, :], ps)

            def mm_chunk(po, xt_sb, k):
                nc.tensor.matmul(
                    po,
                    wt_sb[:, k, :],
                    xt_sb[:, k, :],
                    start=(k == 0),
                    stop=(k == KC - 1),
                    skip_group_check=True,
                )

            def tanh_of(po, nrows, tens):
                t_sb = apool.tile([P, nrows], F16, tag=f"t{tens}")
                nc.scalar.activation(
                    t_sb, po, mybir.ActivationFunctionType.Tanh, bias=bias_sb
                )
                return t_sb

            # 2-stage software pipeline: tile i's matmuls execute
            # interleaved into tile i+1's transpose stream, so PE runs a
            # uniform tr,...,tr,mm pattern with no phase barriers and
            # each cross-engine hop has a full phase of slack.
            prev = None
            for idx, (row0, nrows) in enumerate(tiles):
                g_cnt = nrows // P
                gr0 = row0 // P
                xn1 = natpool.tile([P, g_cnt, DN], F16, tag="xn1")
                nc.sync.dma_start(out=xn1, in_=x1n[:, gr0:gr0 + g_cnt, :DN])
                xn2 = natpool.tile([P, g_cnt, DN], F16, tag="xn2")
                nc.sync.dma_start(out=xn2, in_=x2n[:, gr0:gr0 + g_cnt, :DN])

                xt1_sb = xtpool.tile([P, KC, nrows], F16, tag="xt1")
                xt2_sb = xtpool.tile([P, KC, nrows], F16, tag="xt2")
                # Tail chunks arrive pre-transposed via the DMA XBAR.
                nc.sync.dma_start_transpose(
                    out=xt1_sb[:, K_NAT:, :],
                    in_=x1[row0:row0 + nrows, DN:],
                )
                nc.sync.dma_start_transpose(
                    out=xt2_sb[:, K_NAT:, :],
                    in_=x2[row0:row0 + nrows, DN:],
                )
                if idx == 0:
                    nc.sync.dma_start(
                        out=wt_sb, in_=wt.rearrange("(k p) j -> p k j", p=P)
                    )
                    nc.sync.dma_start(out=bias_sb, in_=bias)
                    nc.sync.dma_start(out=ones_sb, in_=ones)

                cur = dict(row0=row0, nrows=nrows, idx=idx,
                           xt1=xt1_sb, xt2=xt2_sb, po1=None, po2=None)

                if prev is not None:
                    prev["po1"] = opool.tile([P, prev["nrows"]], F32, name="po1", tag="po")
                for k in range(KC):
                    if k < K_NAT:
                        tr_chunk(xn1, xt1_sb, g_cnt, nrows, k, k % 2)
                    if prev is not None:
                        mm_chunk(prev["po1"], prev["xt1"], k)
                    if k == 2:
                        flush_pending()  # sim of tile idx-2 rides here
                if prev is not None:
                    t1 = tanh_of(prev["po1"], prev["nrows"], 0)
                    prev["po2"] = opool.tile([P, prev["nrows"]], F32, name="po2", tag="po")
                for k in range(KC):
                    if k < K_NAT:
                        tr_chunk(xn2, xt2_sb, g_cnt, nrows, k, (k + 1) % 2)
                    if prev is not None:
                        mm_chunk(prev["po2"], prev["xt2"], k)
                if prev is not None:
                    t2 = tanh_of(prev["po2"], prev["nrows"], 1)
                    prod = apool.tile([P, prev["nrows"]], F16, tag="prod")
                    nc.vector.tensor_mul(prod, t1, t2)
                    pending.append((prod, prev["row0"], prev["nrows"], prev["idx"]))
                prev = cur

            # drain last tile
            prev["po1"] = opool.tile([P, prev["nrows"]], F32, name="po1", tag="po")
            for k in range(KC):
                mm_chunk(prev["po1"], prev["xt1"], k)
                if k == 2:
                    flush_pending()
            t1 = tanh_of(prev["po1"], prev["nrows"], 0)
            prev["po2"] = opool.tile([P, prev["nrows"]], F32, name="po2", tag="po")
            for k in range(KC):
                mm_chunk(prev["po2"], prev["xt2"], k)
            t2 = tanh_of(prev["po2"], prev["nrows"], 1)
            prod = apool.tile([P, prev["nrows"]], F16, tag="prod")
            nc.vector.tensor_mul(prod, t1, t2)
            pending.append((prod, prev["row0"], prev["nrows"], prev["idx"]))
            flush_pending()

    nc.compile()
    return nc


_NC_CACHE = None


def _get_module():
    global _NC_CACHE
    if _NC_CACHE is None:
        _NC_CACHE = _build_module()
    return _NC_CACHE


def _prep_inputs(x1, x2, W, b):
    x1 = np.asarray(x1, dtype=np.float16)
    x2 = np.asarray(x2, dtype=np.float16)
    wt = np.ascontiguousarray(np.asarray(W, dtype=np.float16).T)
    bias = np.ascontiguousarray(np.asarray(b, dtype=np.float32).reshape(P, 1))
    ident = np.eye(P, dtype=np.float16)
    ones = np.ones((P, P), dtype=np.float16)
    return [
        {
            "xc": np.concatenate(
                [x1[i * BSH:(i + 1) * BSH], x2[i * BSH:(i + 1) * BSH]], axis=0
            ),
            "wt": wt,
            "bias": bias,
            "ident": ident,
            "ones": ones,
        }
        for i in range(N_CORES)
    ]


def kernel(x1, x2, W, b):
    nc = _get_module()
    in_maps = _prep_inputs(x1, x2, W, b)
    res = run_bass_kernel_spmd(nc, in_maps, core_ids=list(range(N_CORES)))
    return np.concatenate([res.results[i]["out"] for i in range(N_CORES)])


# revision 3
# speedup vs baseline: 1.0006x; 1.0006x over previous
"""Trainium2 Bass kernel: sigmoid(rowdot(tanh(x1@W.T+b), tanh(x2@W.T+b))).

Sharding: pure data-parallel over batch across 8 NeuronCores. Per-core
shapes hardcoded (B=65536 total -> 8192 rows/core, D_IN=1024, D_PROJ=128).
x1/x2 shards are fused into one device tensor "xc" [2*8192, 1024] cast to
fp16 on the host; W.T (fp16), bias (fp32), identity and all-ones (fp16)
are tiny host-precomputed inputs.

fp16 halves HBM traffic vs fp32 (32 MiB/core, ~93 us at the ~358 GB/s
per-core limit) while keeping enough mantissa (11 bits) that end-to-end
max rel err stays ~1e-2 under the 2e-2 gate. With fp32 the kernel was
DMA-bound at ~187 us; at fp16 the bottleneck moves to the PE if it also
does all transposes, so transposes are split:

  - chunks 0..K_NAT-1 of each row-tile load naturally ([128p, g, 768d])
    and are transposed on the PE (fp16 transpose = 1 cyc/row), copied
    PSUM->SBUF by DVE/ACT alternately;
  - chunks K_NAT..7 (columns 768..1024) load via the DMA XBAR transpose
    (dma_start_transpose, 16x128 source tiles, 2-byte dtype) directly
    into the transposed SBUF layout, costing DMA ~14ns/4KiB-tile but
    zero PE/DVE/ACT work.

With K_DMAT=2 both DMA (~99 us) and PE (matmul 55 + transpose 41 +
reduce 3 us) are balanced near their rooflines.

Per-core dataflow per 512-row batch tile (256-row tiles at both ends to
shorten pipeline ramp-in and drain), all-fp16 compute path:
  1. natural x loads + XBAR-transposed tail-chunk loads (SP queue).
  2. PE transpose fp16 -> PSUM for natural chunks; DVE/ACT copy to SBUF.
  3. PE matmul fp16 (1 cyc/row): oT[j,b] += Wt_k.T @ xT_k, fp32 PSUM.
  4. ACT: t = tanh(oT + bias) -> fp16 SBUF.
  5. DVE: prod = t1 * t2 (fp16).
  6. PE: sim = ones.T @ prod -> fp32 PSUM (partition reduction).
  7. ACT sigmoid -> fp32; 2 KiB output DMA on a rotating partition.

Software pipelining (as in the fp32 version): tile i's matmuls are
emitted interleaved into tile i+1's transpose stream (keeps the PE HAM
clock-gate warm, no phase barriers), and tile i's reduce rides inside
tile i+2's transpose phase. PSUM: 5 transpose tiles + 3 matmul banks.
"""

import numpy as np

import concourse.bacc as bacc
import concourse.mybir as mybir
import concourse.tile as tile
from concourse.bass_utils import run_bass_kernel_spmd

N_CORES = 8
B_TOTAL = 65536
BSH = B_TOTAL // N_CORES  # 8192 rows per core
D_IN = 1024
D_PROJ = 128
P = 128
BT = 512                 # batch tile (matmul moving dim)
NBT = BSH // BT          # 16 batch tiles per core
KC = D_IN // P           # 8 contraction chunks
K_DMAT = 2               # chunks per branch loaded via DMA XBAR transpose
K_NAT = KC - K_DMAT      # chunks via natural load + PE transpose
DN = K_NAT * P           # natural-load columns

F32 = mybir.dt.float32
F16 = mybir.dt.float16


def _build_module():
    nc = bacc.Bacc("TRN2", target_bir_lowering=False, debug=False)

    xc = nc.dram_tensor("xc", [2 * BSH, D_IN], F16, kind="ExternalInput").ap()
    x1 = xc[:BSH]
    x2 = xc[BSH:]
    wt = nc.dram_tensor("wt", [D_IN, D_PROJ], F16, kind="ExternalInput").ap()
    bias = nc.dram_tensor("bias", [P, 1], F32, kind="ExternalInput").ap()
    ident = nc.dram_tensor("ident", [P, P], F16, kind="ExternalInput").ap()
    ones = nc.dram_tensor("ones", [P, P], F16, kind="ExternalInput").ap()
    out = nc.dram_tensor("out", [BSH], F32, kind="ExternalOutput").ap()

    outf = out  # [BSH]
    x1n = x1.rearrange("(g p) d -> p g d", p=P)  # [128, BSH//128, D_IN]
    x2n = x2.rearrange("(g p) d -> p g d", p=P)

    with tile.TileContext(nc) as tc:
        with (
            tc.tile_pool(name="consts", bufs=1) as cpool,
            tc.tile_pool(name="xnat", bufs=4) as natpool,
            tc.tile_pool(name="xt", bufs=3) as xtpool,
            tc.tile_pool(name="acts", bufs=2) as apool,
            tc.tile_pool(name="ptr", bufs=5, space="PSUM") as trpool,
            tc.tile_pool(name="po", bufs=3, space="PSUM") as opool,
        ):
            # identity first (32 KiB) -- it gates the first transposes; the
            # 256 KiB W.T load is emitted after the first x-tile loads.
            ident_sb = cpool.tile([P, P], F16, tag="ident")
            nc.sync.dma_start(out=ident_sb, in_=ident)
            wt_sb = cpool.tile([P, KC, D_PROJ], F16, tag="wt")
            bias_sb = cpool.tile([P, 1], F32, tag="bias")
            ones_sb = cpool.tile([P, P], F16, tag="ones")

            # Work list: (row0, nrows). First and last 512-row blocks are
            # split into 256-row subtiles: small first tiles shorten the
            # pipeline ramp-in, small last tiles shorten the drain.
            h = BT // 2
            tiles = [(0, h), (h, h)]
            tiles += [(t * BT, BT) for t in range(1, NBT - 1)]
            last = (NBT - 1) * BT
            tiles += [(last, h), (last + h, h)]

            # Tail of tile i (rowdot reduce + sigmoid + store) is emitted
            # in the middle of tile i+1's transpose phase so PE never
            # waits on the tanh->mul chain.
            pending = []

            def flush_pending():
                while pending:
                    prod_p, row0_p, nr_p, idx_p = pending.pop(0)
                    psim = opool.tile([P, nr_p], F32, name="psim", tag="po")
                    nc.tensor.matmul(
                        psim,
                        ones_sb,
                        prod_p,
                        start=True,
                        stop=True,
                        skip_group_check=True,
                    )
                    sig = apool.tile([P, nr_p], F32, tag="sig")
                    nc.scalar.activation(
                        sig, psim, mybir.ActivationFunctionType.Sigmoid
                    )
                    row = (idx_p * 4) % P  # rotate partition -> spread DMA engines
                    nc.scalar.dma_start(
                        out=outf[row0_p:row0_p + nr_p].rearrange(
                            "(a n) -> a n", a=1
                        ),
                        in_=sig[row:row + 1, :],
                    )

            def tr_chunk(xn, xt_sb, g_cnt, nrows, k, eng):
                ps = trpool.tile([P, nrows], F16, tag="tr")
                for g in range(g_cnt):
                    nc.tensor.transpose(
                        ps[:, g * P:(g + 1) * P],
                        xn[:, g, k * P:(k + 1) * P],
                        ident_sb,
                    )
                if eng == 0:
                    nc.vector.tensor_copy(xt_sb[:, k, :], ps)
                else:
                    nc.scalar.copy(xt_sb[:, k, :], ps)

            def mm_chunk(po, xt_sb, k):
                nc.tensor.matmul(
                    po,
                    wt_sb[:, k, :],
                    xt_sb[:, k, :],
                    start=(k == 0),
                    stop=(k == KC - 1),
                    skip_group_check=True,
                )

            def tanh_of(po, nrows, tens):
                t_sb = apool.tile([P, nrows], F16, tag=f"t{tens}")
                nc.scalar.activation(
                    t_sb, po, mybir.ActivationFunctionType.Tanh, bias=bias_sb
                )
                return t_sb

            # 2-stage software pipeline: tile i's matmuls execute
            # interleaved into tile i+1's transpose stream, so PE runs a
            # uniform tr,...,tr,mm pattern with no phase barriers and
            # each cross-engine hop has a full phase of slack.
            prev = None
            for idx, (row0, nrows) in enumerate(tiles):
                g_cnt = nrows // P
                gr0 = row0 // P
                xn1 = natpool.tile([P, g_cnt, DN], F16, tag="xn1")
                nc.sync.dma_start(out=xn1, in_=x1n[:, gr0:gr0 + g_cnt, :DN])
                xn2 = natpool.tile([P, g_cnt, DN], F16, tag="xn2")
                nc.sync.dma_start(out=xn2, in_=x2n[:, gr0:gr0 + g_cnt, :DN])

                xt1_sb = xtpool.tile([P, KC, nrows], F16, tag="xt1")
                xt2_sb = xtpool.tile([P, KC, nrows], F16, tag="xt2")
                # Tail chunks arrive pre-transposed via the DMA XBAR.
                nc.sync.dma_start_transpose(
                    out=xt1_sb[:, K_NAT:, :],
                    in_=x1[row0:row0 + nrows, DN:],
                )
                nc.sync.dma_start_transpose(
                    out=xt2_sb[:, K_NAT:, :],
                    in_=x2[row0:row0 + nrows, DN:],
                )
                if idx == 0:
                    nc.sync.dma_start(
                        out=wt_sb, in_=wt.rearrange("(k p) j -> p k j", p=P)
                    )
                    nc.sync.dma_start(out=bias_sb, in_=bias)
                    nc.sync.dma_start(out=ones_sb, in_=ones)

                cur = dict(row0=row0, nrows=nrows, idx=idx,
                           xt1=xt1_sb, xt2=xt2_sb, po1=None, po2=None)

                if prev is not None:
                    prev["po1"] = opool.tile([P, prev["nrows"]], F32, name="po1", tag="po")
                for k in range(KC):
                    if k < K_NAT:
                        tr_chunk(xn1, xt1_sb, g_cnt, nrows, k, k % 2)
                    if prev is not None:
                        mm_chunk(prev["po1"], prev["xt1"], k)
                    if k == 2:
                        flush_pending()  # sim of tile idx-2 rides here
                if prev is not None:
                    t1 = tanh_of(prev["po1"], prev["nrows"], 0)
                    prev["po2"] = opool.tile([P, prev["nrows"]], F32, name="po2", tag="po")
                for k in range(KC):
                    if k < K_NAT:
                        tr_chunk(xn2, xt2_sb, g_cnt, nrows, k, (k + 1) % 2)
                    if prev is not None:
                        mm_chunk(prev["po2"], prev["xt2"], k)
                if prev is not None:
                    t2 = tanh_of(prev["po2"], prev["nrows"], 1)
                    prod = apool.tile([P, prev["nrows"]], F16, tag="prod")
                    nc.vector.tensor_mul(prod, t1, t2)
                    pending.append((prod, prev["row0"], prev["nrows"], prev["idx"]))
                prev = cur

            # drain last tile
            prev["po1"] = opool.tile([P, prev["nrows"]], F32, name="po1", tag="po")
            for k in range(KC):
                mm_chunk(prev["po1"], prev["xt1"], k)
                if k == 2:
                    flush_pending()
            t1 = tanh_of(prev["po1"], prev["nrows"], 0)
            prev["po2"] = opool.tile([P, prev["nrows"]], F32, name="po2", tag="po")
            for k in range(KC):
                mm_chunk(prev["po2"], prev["xt2"], k)
            t2 = tanh_of(prev["po2"], prev["nrows"], 1)
            prod = apool.tile([P, prev["nrows"]], F16, tag="prod")
            nc.vector.tensor_mul(prod, t1, t2)
            pending.append((prod, prev["row0"], prev["nrows"], prev["idx"]))
            flush_pending()

    nc.compile()
    return nc


_NC_CACHE = None


def _get_module():
    global _NC_CACHE
    if _NC_CACHE is None:
        _NC_CACHE = _build_module()
    return _NC_CACHE


def _prep_inputs(x1, x2, W, b):
    x1 = np.asarray(x1, dtype=np.float16)
    x2 = np.asarray(x2, dtype=np.float16)
    wt = np.ascontiguousarray(np.asarray(W, dtype=np.float16).T)
    bias = np.ascontiguousarray(np.asarray(b, dtype=np.float32).reshape(P, 1))
    ident = np.eye(P, dtype=np.float16)
    ones = np.ones((P, P), dtype=np.float16)
    return [
        {
            "xc": np.concatenate(
                [x1[i * BSH:(i + 1) * BSH], x2[i * BSH:(i + 1) * BSH]], axis=0
            ),
            "wt": wt,
            "bias": bias,
            "ident": ident,
            "ones": ones,
        }
        for i in range(N_CORES)
    ]


def kernel(x1, x2, W, b):
    nc = _get_module()
    in_maps = _prep_inputs(x1, x2, W, b)
    res = run_bass_kernel_spmd(nc, in_maps, core_ids=list(range(N_CORES)))
    return np.concatenate([res.results[i]["out"] for i in range(N_CORES)])
